# revision 30
# baseline (speedup 1.0000x reference)
"""Trainium2 Bass kernel for causal self-attention (B=4, T=2048, C=2048, H=16).

Sharding: 8 cores = DP4 (batch) x TP2 (8 heads each). v2 design:
  Phase V   v = x @ Wv + bv in [t, j] layout (stationary = xT tiles).
  Per head h (0..7):
    QK  q'_h, k'_h = (x @ Wqk)^T in [j, t] layout, kept in SBUF
        (q scaled by 1/sqrt(D) host-side; bias via DVE column add).
    ATT per 512-wide query chunk: s = k'^T q' (PE) -> exp (ACT)
        -> av + ones-den (PE, PSUM accumulation) with diagonal blocks
        narrowed to the causal range; the triangle handled by a 0/1
        mask multiply on DVE after exp. y'_h = po * recip_approx(den).
        Software-pipelined: s-matmuls run 2 blocks ahead of av-matmuls
        so the PE never waits on the ACT exp.
    y'_h staged to DRAM; per-2-head AllGather pairs exchange y so both
    cores hold all 16 heads (overlapped with later heads' compute).
  Phase P   out[:, core_half] = y'^T @ Wp + bp over all 16 heads read
        from the gathered copy, streamed out per 128-row block.
        No tail collective.

All matmuls bf16 with fp32 PSUM accumulation; softmax in fp32 on ACT/DVE.
"""
import math
import numpy as np
import ml_dtypes

import concourse.bass as bass
import concourse.bacc as bacc
import concourse.mybir as mybir
import concourse.tile as tile

F32 = mybir.dt.float32
BF16 = mybir.dt.bfloat16
AF = mybir.ActivationFunctionType

D = 128          # head dim (fixed: partition size)
N_CORES = 8
PAIRS = [[0, 1], [2, 3], [4, 5], [6, 7]]
# proj contraction order over global heads: AllGather groups 0-1
# (staged in SBUF during the head loop) first, then group 2, then the
# last group 3 so its exchange + staging overlap the proj phase start.
PROJ_ORDER = [0, 1, 2, 3, 8, 9, 10, 11, 4, 5, 12, 13, 6, 7, 14, 15]


class Cfg:
    def __init__(self, T=2048, H_TOT=16, HPC=8, B=4):
        self.T = T                    # sequence length
        self.H_TOT = H_TOT            # total heads
        self.HPC = HPC                # heads per core
        self.B = B
        self.C = H_TOT * D            # model dim
        self.CP = HPC * D             # per-core head cols
        self.TCH = 512                # query chunk width
        assert T % self.TCH == 0 and T % D == 0


def build_kernel(cfg: Cfg):
    T, C, CP, HPC, TCH = cfg.T, cfg.C, cfg.CP, cfg.HPC, cfg.TCH
    NC_CH = C // D                # c-chunks (contraction)
    NTB = T // D                  # t-blocks
    NIC = T // TCH                # query chunks per head
    NTR = T // 512                # t-ranges for qk moving dim
    NDB = TCH // D                # diagonal blocks per chunk
    COLS = C // 2                 # output columns owned per core
    NPR = COLS // 512             # proj n-ranges

    nc = bacc.Bacc()
    xT = nc.declare_dram_parameter("xT", [C, T], BF16, isOutput=False)
    wqk = nc.declare_dram_parameter("wqk", [C, 2 * CP], BF16, isOutput=False)
    wv = nc.declare_dram_parameter("wv", [C, CP], BF16, isOutput=False)
    wp = nc.declare_dram_parameter("wp", [C, COLS], BF16, isOutput=False)
    bqk = nc.declare_dram_parameter("bqk", [D, 2 * HPC], F32, isOutput=False)
    bvb = nc.declare_dram_parameter("bvb", [D, CP], F32, isOutput=False)
    bpb = nc.declare_dram_parameter("bpb", [D, COLS], F32, isOutput=False)
    tri = nc.declare_dram_parameter("tri", [D, D], BF16, isOutput=False)
    out_ext = nc.declare_dram_parameter("out", [T, COLS], F32, isOutput=True)

    y_dram = nc.dram_tensor("y_dram", [HPC, D, T], BF16)
    ag_out = nc.dram_tensor("ag_out", [HPC // 2, 2, 2, D, T], BF16)

    with tile.TileContext(nc) as tc:
        with (
            tc.tile_pool(name="const", bufs=1) as constp,
        ):
            ypa_ctx = tc.tile_pool(name="ypa", bufs=1)
            ypa = ypa_ctx.__enter__()
            vres_ctx = tc.tile_pool(name="vres", bufs=1)
            vres = vres_ctx.__enter__()
            # consts go on the Activation HWDGE queue: it is idle at start,
            # so these issue in parallel with the xT loads on the SP queue.
            bqk_t = constp.tile([D, 2 * HPC], F32, name="bqk_t")
            nc.scalar.dma_start(bqk_t[:], bqk[:, :])
            bv_t = constp.tile([D, CP], F32, name="bv_t")
            nc.scalar.dma_start(bv_t[:], bvb[:, :])
            tri_t = constp.tile([D, D], BF16, name="tri_t")
            nc.scalar.dma_start(tri_t[:], tri[:, :])
            ones_sq = constp.tile([D, D], BF16, name="ones_sq")
            nc.vector.memset(ones_sq[:], 1.0)

            # resident xT tiles first (QK0 is paced by their arrival),
            # then wv (only needed once the v-phase starts, ~30us in).
            xtp_ctx = tc.tile_pool(name="xtp", bufs=1)
            xtp = xtp_ctx.__enter__()
            wqkp_ctx = tc.tile_pool(name="wqkp", bufs=2)
            wqkp = wqkp_ctx.__enter__()
            qkst_ctx = tc.tile_pool(name="qkst", bufs=2)
            qkst = qkst_ctx.__enter__()
            wvp_ctx = tc.tile_pool(name="wvp", bufs=1)
            wvp = wvp_ctx.__enter__()

            def load_wts(h, eng=None):
                """One fused DMA for all 16 c-chunks of head h's q|k cols."""
                wt_all = wqkp.tile([D, NC_CH * 2 * D], BF16, name="wt",
                                   tag="wt")
                (eng or nc.sync).dma_start(
                    wt_all[:],
                    wqk[:, h * 2 * D:(h + 1) * 2 * D].rearrange(
                        "(c p) f -> p c f", p=D))
                return [wt_all[:, c * 2 * D:(c + 1) * 2 * D]
                        for c in range(NC_CH)]

            # head-0 weights on the Activation queue (parallel with xT)
            wts0 = load_wts(0, eng=nc.scalar)
            # xT in 4 fused DMAs of 4 c-chunks each on the SP queue: cheap
            # to issue, and QK0's chains consume them group by group.
            xt = []
            for g in range(4):
                xg = xtp.tile([D, 4 * T], BF16, name=f"xtg{g}")
                nc.sync.dma_start(
                    xg[:], xT[g * 4 * D:(g + 1) * 4 * D, :].rearrange(
                        "(c p) f -> p c f", p=D))
                for cc in range(4):
                    xt.append(xg[:, cc * T:(cc + 1) * T])
            wv_t = []
            for g in range(2):
                wg = wvp.tile([D, 8 * CP], BF16, name=f"wvg{g}")
                nc.sync.dma_start(
                    wg[:], wv[g * 8 * D:(g + 1) * 8 * D, :].rearrange(
                        "(c p) f -> p c f", p=D))
                for cc in range(8):
                    wv_t.append(wg[:, cc * CP:(cc + 1) * CP])

            def emit_qk(h, wts):
                """QK chains + bias for head h -> (qp, kp) bf16 in SBUF."""
                qk_sb = qkst.tile([D, 2 * T], BF16, name="qk_sb", tag="qk")
                qp = qk_sb[:, 0:T]
                kp = qk_sb[:, T:2 * T]
                with tc.tile_pool(name="pq", bufs=8, space="PSUM") as pqp:
                    for half in range(2):   # 0 = q, 1 = k
                        ps = [pqp.tile([D, 512], F32, name="pq", tag="pq")
                              for _ in range(NTR)]
                        for c in range(NC_CH):
                            for tr in range(NTR):
                                nc.tensor.matmul(
                                    ps[tr][:],
                                    wts[c][:, half * D:(half + 1) * D],
                                    xt[c][:, tr * 512:(tr + 1) * 512],
                                    start=(c == 0), stop=(c == NC_CH - 1))
                        dst = qp if half == 0 else kp
                        for tr in range(NTR):
                            nc.vector.tensor_scalar_add(
                                dst[:, tr * 512:(tr + 1) * 512], ps[tr][:],
                                bqk_t[:, 2 * h + half:2 * h + half + 1])
                return qp, kp

            # QK for head 0 runs first: it only needs xT, so the PE can
            # start as soon as the first xT tiles land.
            qk_cur = emit_qk(0, wts0)

            # ---------------- Phase V: v = x @ Wv + bv ----------------
            # 4 t-block chains advance together per c-step so the PE keeps
            # pace with the wv input DMA stream instead of stalling
            # in-order behind one chain.
            v_sb = [None] * NTB
            with tc.tile_pool(name="pv", bufs=8, space="PSUM") as pvp:
                NVR = CP // 512
                GRP = 4
                for tb0 in range(0, NTB, GRP):
                    ps = {}
                    for tb in range(tb0, tb0 + GRP):
                        for vr in range(NVR):
                            ps[tb, vr] = pvp.tile([D, 512], F32, name="pv",
                                                  tag="pv")
                    for c in range(NC_CH):
                        for tb in range(tb0, tb0 + GRP):
                            for vr in range(NVR):
                                nc.tensor.matmul(
                                    ps[tb, vr][:],
                                    xt[c][:, tb * D:(tb + 1) * D],
                                    wv_t[c][:, vr * 512:(vr + 1) * 512],
                                    start=(c == 0), stop=(c == NC_CH - 1))
                    for tb in range(tb0, tb0 + GRP):
                        vt = vres.tile([D, CP], BF16, name=f"v{tb}")
                        for vr in range(NVR):
                            nc.vector.tensor_add(
                                vt[:, vr * 512:(vr + 1) * 512], ps[tb, vr][:],
                                bv_t[:, vr * 512:(vr + 1) * 512])
                        v_sb[tb] = vt
            wvp_ctx.__exit__(None, None, None)
            yres_ctx = tc.tile_pool(name="yres", bufs=2)
            yres = yres_ctx.__enter__()

            # ---------------- per-head attention (QK h+1 emitted first,
            # so its bias-add latency hides behind attention h) ---------
            scale = 1.0  # q pre-scaled host-side
            agy = {}
            if True:
                for h in range(HPC):
                    qp, kp = qk_cur
                    if h + 1 < HPC:
                        wts = load_wts(h + 1)
                        qk_cur = emit_qk(h + 1, wts)

                    # --- attention for head h ---
                    yt = yres.tile([D, T], BF16, name="yt", tag="yt")
                    # flat block list across chunks: (ic, tk, joff)
                    work = []
                    for ic in range(NIC):
                        for tk in range(NDB * ic):
                            work.append((ic, tk, -1))
                        for j in range(NDB):
                            work.append((ic, NDB * ic + j, j))
                    n = len(work)
                    att_tiles = [None] * n
                    po_t = {}
                    pd_t = {}
                    with (
                        tc.tile_pool(name="ps_s", bufs=3, space="PSUM") as ps_s,
                        tc.tile_pool(name="attp", bufs=4) as attp,
                        tc.tile_pool(name="ps_o", bufs=2, space="PSUM") as ps_o,
                        tc.tile_pool(name="ps_d", bufs=2, space="PSUM") as ps_d,
                        tc.tile_pool(name="normp", bufs=2) as normp,
                    ):
                        def emit_s(i):
                            ic, tk, j = work[i]
                            ti0 = ic * TCH
                            w = TCH if j < 0 else TCH - j * D
                            off = 0 if j < 0 else j * D
                            sp = ps_s.tile([D, TCH], F32, name="sp", tag="sp")
                            nc.tensor.matmul(
                                sp[:, 0:w], kp[:, tk * D:(tk + 1) * D],
                                qp[:, ti0 + off:ti0 + TCH],
                                start=True, stop=True)
                            att = attp.tile([D, TCH], BF16, name="att",
                                            tag="att")
                            nc.scalar.activation(att[:, 0:w], sp[:, 0:w],
                                                 AF.Exp, bias=0.0, scale=scale)
                            if j >= 0:
                                nc.vector.tensor_mul(
                                    att[:, 0:D], att[:, 0:D], tri_t[:])
                            att_tiles[i] = att

                        def emit_av(i):
                            ic, tk, j = work[i]
                            w = TCH if j < 0 else TCH - j * D
                            off = 0 if j < 0 else j * D
                            first = (j == 0 if ic == 0 else tk == 0)
                            last = (j == NDB - 1)
                            if first:
                                po_t[ic] = ps_o.tile([D, TCH], F32, name="po",
                                                     tag="po")
                                pd_t[ic] = ps_d.tile([D, TCH], F32, name="pd",
                                                     tag="pd")
                            att = att_tiles[i]
                            nc.tensor.matmul(
                                po_t[ic][:, off:TCH],
                                v_sb[tk][:, h * D:(h + 1) * D], att[:, 0:w],
                                start=first, stop=last,
                                skip_group_check=True)
                            nc.tensor.matmul(
                                pd_t[ic][:, off:TCH], ones_sq[:], att[:, 0:w],
                                start=first, stop=last,
                                skip_group_check=True)
                            att_tiles[i] = None
                            if last:
                                ti0 = ic * TCH
                                rec = normp.tile([D, TCH], F32, name="rec",
                                                 tag="rec")
                                nc.vector.reciprocal_approx_fast(
                                    rec[:], pd_t[ic][:])
                                nc.vector.tensor_mul(
                                    yt[:, ti0:ti0 + TCH], po_t[ic][:], rec[:])

                        emit_s(0)
                        if n > 1:
                            emit_s(1)
                        for i in range(n):
                            if i + 2 < n:
                                emit_s(i + 2)
                            emit_av(i)
                    nc.sync.dma_start(y_dram[h, :, :], yt[:])
                    # exchange pairs of heads as soon as both are staged
                    if h % 2 == 1:
                        g = h // 2
                        nc.gpsimd.collective_compute(
                            "AllGather",
                            mybir.AluOpType.bypass,
                            ins=[y_dram[2 * g:2 * g + 2, :, :]],
                            outs=[ag_out[g, :, :, :, :]],
                            replica_groups=PAIRS,
                        )
                        # stage gathered heads of groups 0-1 into SBUF
                        # now (groups 2-3 are staged at proj start, when
                        # the xT tiles have been freed). Rank order is
                        # core-id order: identical layout on both cores.
                        if g < 2:
                            ypt = ypa.tile([D, 4 * T], BF16, name=f"agg{g}")
                            nc.sync.dma_start(
                                ypt[:],
                                ag_out[g].rearrange("r h p f -> p r h f"))
                            for r in range(2):
                                for hh in range(2):
                                    gh = r * 8 + 2 * g + hh
                                    agy[gh] = ypt[:, (2 * r + hh) * T:
                                                  (2 * r + hh + 1) * T]

            # ---------------- Phase P: proj ----------------
            yres_ctx.__exit__(None, None, None)
            qkst_ctx.__exit__(None, None, None)
            wqkp_ctx.__exit__(None, None, None)
            xtp_ctx.__exit__(None, None, None)
            vres_ctx.__exit__(None, None, None)
            with (
                tc.tile_pool(name="ypb", bufs=1) as ypb,
                tc.tile_pool(name="wpp", bufs=1) as wpp,
                tc.tile_pool(name="pp", bufs=4, space="PSUM") as ppp,
                tc.tile_pool(name="post", bufs=3) as post,
            ):
                # wp weights stream on the SP queue (4 fused DMAs); the
                # group 2-3 head staging rides the Activation queue so the
                # two issue in parallel and the AG3-gated transfer never
                # blocks the weight stream.
                def stage(g):
                    ypt = ypb.tile([D, 4 * T], BF16, name=f"agg{g}")
                    nc.scalar.dma_start(
                        ypt[:], ag_out[g].rearrange("r h p f -> p r h f"))
                    for r in range(2):
                        for hh in range(2):
                            gh = r * 8 + 2 * g + hh
                            agy[gh] = ypt[:, (2 * r + hh) * T:
                                          (2 * r + hh + 1) * T]
                stage(2)
                stage(3)
                bp_t = constp.tile([D, COLS], F32, name="bp_t")
                nc.scalar.dma_start(bp_t[:], bpb[:, :])
                wp_t = []
                for g in range(4):
                    wg = wpp.tile([D, 4 * COLS], BF16, name=f"wpg{g}")
                    nc.sync.dma_start(
                        wg[:], wp[g * 4 * D:(g + 1) * 4 * D, :].rearrange(
                            "(c p) f -> p c f", p=D))
                    for cc in range(4):
                        wp_t.append(wg[:, cc * COLS:(cc + 1) * COLS])
                ysrc = [agy[gh] for gh in PROJ_ORDER]
                for tb in range(NTB):
                    ps = [ppp.tile([D, 512], F32, name="pp", tag="pp")
                          for _ in range(NPR)]
                    for ci in range(NC_CH):
                        for nr in range(NPR):
                            nc.tensor.matmul(
                                ps[nr][:], ysrc[ci][:, tb * D:(tb + 1) * D],
                                wp_t[ci][:, nr * 512:(nr + 1) * 512],
                                start=(ci == 0), stop=(ci == NC_CH - 1))
                    st = post.tile([D, COLS], F32, name="pst", tag="pst")
                    for nr in range(NPR):
                        nc.vector.tensor_add(
                            st[:, nr * 512:(nr + 1) * 512], ps[nr][:],
                            bp_t[:, nr * 512:(nr + 1) * 512])
                    nc.sync.dma_start(out_ext[tb * D:(tb + 1) * D, :], st[:])
            ypa_ctx.__exit__(None, None, None)
    nc.finalize()
    return nc


def _prep_inputs(cfg: Cfg, x, w_attn, b_attn, w_proj, b_proj):
    """Host-side shard/cast. Returns in_maps (list of dicts per core)."""
    T, C, CP, HPC = cfg.T, cfg.C, cfg.CP, cfg.HPC
    bf = ml_dtypes.bfloat16
    qscale = 1.0 / math.sqrt(D)
    wq = w_attn[:, 0:C] * qscale
    wk = w_attn[:, C:2 * C]
    wvf = w_attn[:, 2 * C:3 * C]
    bq = b_attn[0:C] * qscale
    bk, bvf = b_attn[C:2 * C], b_attn[2 * C:3 * C]

    # 0/1 lower-triangle keep mask for the diagonal 128-block:
    # att[p=t_k, f=t_q] visible iff f >= p
    f = np.arange(D)[None, :]
    p = np.arange(D)[:, None]
    tri = (f >= p).astype(bf)

    in_maps = []
    for core in range(N_CORES):
        b = core // 2
        g = core % 2
        h0 = g * HPC * D            # first col of this head group
        sl = slice(h0, h0 + CP)
        xTc = np.ascontiguousarray(x[b].T).astype(bf)
        wqk_cols = []
        for h in range(HPC):
            hs = slice(h0 + h * D, h0 + (h + 1) * D)
            wqk_cols.append(wq[:, hs])
            wqk_cols.append(wk[:, hs])
        wqk_c = np.concatenate(wqk_cols, axis=1).astype(bf)
        wv_c = wvf[:, sl].astype(bf)
        # proj: rows in PROJ_ORDER of global heads; cols = core's half
        csl = slice(g * (C // 2), (g + 1) * (C // 2))
        wp_c = np.concatenate(
            [w_proj[gh * D:(gh + 1) * D, csl] for gh in PROJ_ORDER],
            axis=0).astype(bf)
        bqk_cols = []
        for h in range(HPC):
            hs = slice(h0 + h * D, h0 + (h + 1) * D)
            bqk_cols.append(bq[hs])
            bqk_cols.append(bk[hs])
        bqk_c = np.ascontiguousarray(np.stack(bqk_cols, axis=1)).astype(
            np.float32)
        in_maps.append({
            "xT": xTc,
            "wqk": wqk_c,
            "wv": wv_c,
            "wp": wp_c,
            "bqk": bqk_c,
            "bvb": np.broadcast_to(
                bvf[sl].astype(np.float32), (D, CP)).copy(),
            "bpb": np.broadcast_to(
                b_proj[csl].astype(np.float32), (D, C // 2)).copy(),
            "tri": tri,
        })
    return in_maps


_CFG = Cfg()


def kernel(x, w_attn, b_attn, w_proj, b_proj, _trace=False, _cfg=None):
    from concourse.bass_utils import run_bass_kernel_spmd
    cfg = _cfg or _CFG
    x = np.asarray(x, dtype=np.float32)
    w_attn = np.asarray(w_attn, dtype=np.float32)
    b_attn = np.asarray(b_attn, dtype=np.float32)
    w_proj = np.asarray(w_proj, dtype=np.float32)
    b_proj = np.asarray(b_proj, dtype=np.float32)

    in_maps = _prep_inputs(cfg, x, w_attn, b_attn, w_proj, b_proj)
    nc = build_kernel(cfg)
    res = run_bass_kernel_spmd(nc, in_maps, list(range(N_CORES)), trace=_trace)
    outs = []
    for b in range(cfg.B):
        left = res.results[2 * b]["out"]
        right = res.results[2 * b + 1]["out"]
        outs.append(np.concatenate([left, right], axis=1))
    full = np.stack(outs, axis=0).astype(np.float32)
    if _trace:
        kernel.last_exec_time_ns = res.exec_time_ns
        kernel.last_mean_exec_time_ns = res.mean_exec_time_ns
        kernel.last_scope_times = res.per_core_scope_times
    return full


# revision 39
# speedup vs baseline: 1.0658x; 1.0658x over previous
"""Trainium2 Bass kernel for causal self-attention (B=4, T=2048, C=2048, H=16).

Sharding: 8 cores = DP4 (batch) x TP2 (8 heads each). v2 design:
  Phase V   v = x @ Wv + bv in [t, j] layout (stationary = xT tiles).
  Per head h (0..7):
    QK  q'_h, k'_h = (x @ Wqk)^T in [j, t] layout, kept in SBUF
        (q scaled by 1/sqrt(D) host-side; bias via DVE column add).
    ATT per 512-wide query chunk: s = k'^T q' (PE) -> exp (ACT)
        -> av + ones-den (PE, PSUM accumulation) with diagonal blocks
        narrowed to the causal range; the triangle handled by a 0/1
        mask multiply on DVE after exp. y'_h = po * recip_approx(den).
        Software-pipelined: s-matmuls run 2 blocks ahead of av-matmuls
        so the PE never waits on the ACT exp.
    y'_h staged to DRAM; per-2-head AllGather pairs exchange y so both
    cores hold all 16 heads (overlapped with later heads' compute).
  Phase P   out[:, core_half] = y'^T @ Wp + bp over all 16 heads read
        from the gathered copy, streamed out per 128-row block.
        No tail collective.

All matmuls bf16 with fp32 PSUM accumulation; softmax in fp32 on ACT/DVE.
"""
import math
import numpy as np
import ml_dtypes

import concourse.bass as bass
import concourse.bacc as bacc
import concourse.mybir as mybir
import concourse.tile as tile

F32 = mybir.dt.float32
BF16 = mybir.dt.bfloat16
AF = mybir.ActivationFunctionType

D = 128          # head dim (fixed: partition size)
N_CORES = 8
PAIRS = [[0, 1], [2, 3], [4, 5], [6, 7]]
# proj contraction order over global heads: AllGather groups 0-1
# (staged in SBUF during the head loop) first, then group 2, then the
# per-head exchanges of heads 6 and 7 last so they overlap proj start.
PROJ_ORDER = [0, 1, 2, 3, 8, 9, 10, 11, 4, 5, 12, 13, 6, 14, 7, 15]


class Cfg:
    def __init__(self, T=2048, H_TOT=16, HPC=8, B=4):
        self.T = T                    # sequence length
        self.H_TOT = H_TOT            # total heads
        self.HPC = HPC                # heads per core
        self.B = B
        self.C = H_TOT * D            # model dim
        self.CP = HPC * D             # per-core head cols
        self.TCH = 512                # query chunk width
        assert T % self.TCH == 0 and T % D == 0


def build_kernel(cfg: Cfg):
    T, C, CP, HPC, TCH = cfg.T, cfg.C, cfg.CP, cfg.HPC, cfg.TCH
    NC_CH = C // D                # c-chunks (contraction)
    NTB = T // D                  # t-blocks
    NIC = T // TCH                # query chunks per head
    NTR = T // 512                # t-ranges for qk moving dim
    NDB = TCH // D                # diagonal blocks per chunk
    COLS = C // 2                 # output columns owned per core
    NPR = COLS // 512             # proj n-ranges

    nc = bacc.Bacc()
    xT = nc.declare_dram_parameter("xT", [C, T], BF16, isOutput=False)
    wqk = nc.declare_dram_parameter("wqk", [C, 2 * CP], BF16, isOutput=False)
    wv = nc.declare_dram_parameter("wv", [C, CP], BF16, isOutput=False)
    wp = nc.declare_dram_parameter("wp", [C, COLS], BF16, isOutput=False)
    bqk = nc.declare_dram_parameter("bqk", [D, 2 * HPC], F32, isOutput=False)
    bvb = nc.declare_dram_parameter("bvb", [D, CP], F32, isOutput=False)
    bpb = nc.declare_dram_parameter("bpb", [D, COLS], F32, isOutput=False)
    tri = nc.declare_dram_parameter("tri", [D, D], BF16, isOutput=False)
    out_ext = nc.declare_dram_parameter("out", [T, COLS], F32, isOutput=True)

    y_dram = nc.dram_tensor("y_dram", [HPC, D, T], BF16)
    ag_out = nc.dram_tensor("ag_out", [HPC // 2, 2, 2, D, T], BF16)
    ag6 = nc.dram_tensor("ag6", [2, D, T], BF16)
    ag7 = nc.dram_tensor("ag7", [2, D, T], BF16)

    with tile.TileContext(nc) as tc:
        with (
            tc.tile_pool(name="const", bufs=1) as constp,
        ):
            ypa_ctx = tc.tile_pool(name="ypa", bufs=1)
            ypa = ypa_ctx.__enter__()
            vres_ctx = tc.tile_pool(name="vres", bufs=1)
            vres = vres_ctx.__enter__()
            # consts go on the Activation HWDGE queue: it is idle at start,
            # so these issue in parallel with the xT loads on the SP queue.
            bqk_t = constp.tile([D, 2 * HPC], F32, name="bqk_t")
            nc.scalar.dma_start(bqk_t[:], bqk[:, :])
            bv_t = constp.tile([D, CP], F32, name="bv_t")
            nc.scalar.dma_start(bv_t[:], bvb[:, :])
            tri_t = constp.tile([D, D], BF16, name="tri_t")
            nc.scalar.dma_start(tri_t[:], tri[:, :])
            ones_sq = constp.tile([D, D], BF16, name="ones_sq")
            nc.vector.memset(ones_sq[:], 1.0)

            # resident xT tiles first (QK0 is paced by their arrival),
            # then wv (only needed once the v-phase starts, ~30us in).
            xtp_ctx = tc.tile_pool(name="xtp", bufs=1)
            xtp = xtp_ctx.__enter__()
            wqkp_ctx = tc.tile_pool(name="wqkp", bufs=2)
            wqkp = wqkp_ctx.__enter__()
            qkst_ctx = tc.tile_pool(name="qkst", bufs=2)
            qkst = qkst_ctx.__enter__()
            wvp_ctx = tc.tile_pool(name="wvp", bufs=1)
            wvp = wvp_ctx.__enter__()

            def load_wts(h, eng=None, fused=True):
                """Head h's q|k weight cols -> one [D, 16*256] tile.
                fused: single strided DMA (cheap issue, ~18us transfer —
                fine when prefetched a head ahead). Non-fused: 16 per-chunk
                DMAs for incremental arrival (startup)."""
                wt_all = wqkp.tile([D, NC_CH * 2 * D], BF16, name="wt",
                                   tag="wt")
                eng = eng or nc.sync
                if fused:
                    eng.dma_start(
                        wt_all[:],
                        wqk[:, h * 2 * D:(h + 1) * 2 * D].rearrange(
                            "(c p) f -> p c f", p=D))
                else:
                    for c in range(NC_CH):
                        eng.dma_start(
                            wt_all[:, c * 2 * D:(c + 1) * 2 * D],
                            wqk[c * D:(c + 1) * D,
                                h * 2 * D:(h + 1) * 2 * D])
                return [wt_all[:, c * 2 * D:(c + 1) * 2 * D]
                        for c in range(NC_CH)]

            # head-0 weights per-chunk on the Activation queue: they land
            # incrementally, in parallel with (and faster than) the xT
            # stream on the SP queue that pances QK0.
            wts0 = load_wts(0, eng=nc.scalar, fused=False)
            xt = []
            for c in range(NC_CH):
                t = xtp.tile([D, T], BF16, name=f"xt{c}")
                nc.sync.dma_start(t[:], xT[c * D:(c + 1) * D, :])
                xt.append(t)
            wv_t = []
            for c in range(NC_CH):
                w = wvp.tile([D, CP], BF16, name=f"wv{c}")
                nc.sync.dma_start(w[:], wv[c * D:(c + 1) * D, :])
                wv_t.append(w)

            def emit_qk(h, wts):
                """QK chains + bias for head h -> (qp, kp) bf16 in SBUF."""
                qk_sb = qkst.tile([D, 2 * T], BF16, name="qk_sb", tag="qk")
                qp = qk_sb[:, 0:T]
                kp = qk_sb[:, T:2 * T]
                with tc.tile_pool(name="pq", bufs=8, space="PSUM") as pqp:
                    for half in range(2):   # 0 = q, 1 = k
                        ps = [pqp.tile([D, 512], F32, name="pq", tag="pq")
                              for _ in range(NTR)]
                        for c in range(NC_CH):
                            for tr in range(NTR):
                                nc.tensor.matmul(
                                    ps[tr][:],
                                    wts[c][:, half * D:(half + 1) * D],
                                    xt[c][:, tr * 512:(tr + 1) * 512],
                                    start=(c == 0), stop=(c == NC_CH - 1))
                        dst = qp if half == 0 else kp
                        for tr in range(NTR):
                            nc.vector.tensor_scalar_add(
                                dst[:, tr * 512:(tr + 1) * 512], ps[tr][:],
                                bqk_t[:, 2 * h + half:2 * h + half + 1])
                return qp, kp

            # QK for head 0 runs first: it only needs xT, so the PE can
            # start as soon as the first xT tiles land.
            qk_cur = emit_qk(0, wts0)

            # ---------------- Phase V: v = x @ Wv + bv ----------------
            # 4 t-block chains advance together per c-step so the PE keeps
            # pace with the wv input DMA stream instead of stalling
            # in-order behind one chain.
            v_sb = [None] * NTB
            with tc.tile_pool(name="pv", bufs=8, space="PSUM") as pvp:
                NVR = CP // 512
                GRP = 4
                for tb0 in range(0, NTB, GRP):
                    ps = {}
                    for tb in range(tb0, tb0 + GRP):
                        for vr in range(NVR):
                            ps[tb, vr] = pvp.tile([D, 512], F32, name="pv",
                                                  tag="pv")
                    for c in range(NC_CH):
                        for tb in range(tb0, tb0 + GRP):
                            for vr in range(NVR):
                                nc.tensor.matmul(
                                    ps[tb, vr][:],
                                    xt[c][:, tb * D:(tb + 1) * D],
                                    wv_t[c][:, vr * 512:(vr + 1) * 512],
                                    start=(c == 0), stop=(c == NC_CH - 1))
                    for tb in range(tb0, tb0 + GRP):
                        vt = vres.tile([D, CP], BF16, name=f"v{tb}")
                        for vr in range(NVR):
                            nc.vector.tensor_add(
                                vt[:, vr * 512:(vr + 1) * 512], ps[tb, vr][:],
                                bv_t[:, vr * 512:(vr + 1) * 512])
                        v_sb[tb] = vt
            wvp_ctx.__exit__(None, None, None)
            yres_ctx = tc.tile_pool(name="yres", bufs=2)
            yres = yres_ctx.__enter__()

            # ---------------- per-head attention (QK h+1 emitted first,
            # so its bias-add latency hides behind attention h; weight
            # loads prefetched one further head ahead) -------------------
            scale = 1.0  # q pre-scaled host-side
            agy = {}
            wts_next = load_wts(1)
            if True:
                for h in range(HPC):
                    qp, kp = qk_cur
                    if h + 1 < HPC:
                        qk_cur = emit_qk(h + 1, wts_next)
                    if h + 2 < HPC:
                        wts_next = load_wts(h + 2)

                    # --- attention for head h ---
                    yt = yres.tile([D, T], BF16, name="yt", tag="yt")
                    # flat block list across chunks: (ic, tk, joff)
                    work = []
                    for ic in range(NIC):
                        for tk in range(NDB * ic):
                            work.append((ic, tk, -1))
                        for j in range(NDB):
                            work.append((ic, NDB * ic + j, j))
                    n = len(work)
                    att_tiles = [None] * n
                    po_t = {}
                    pd_t = {}
                    with (
                        tc.tile_pool(name="ps_s", bufs=3, space="PSUM") as ps_s,
                        tc.tile_pool(name="attp", bufs=4) as attp,
                        tc.tile_pool(name="ps_o", bufs=2, space="PSUM") as ps_o,
                        tc.tile_pool(name="ps_d", bufs=2, space="PSUM") as ps_d,
                        tc.tile_pool(name="normp", bufs=2) as normp,
                    ):
                        def emit_s(i):
                            ic, tk, j = work[i]
                            ti0 = ic * TCH
                            w = TCH if j < 0 else TCH - j * D
                            off = 0 if j < 0 else j * D
                            sp = ps_s.tile([D, TCH], F32, name="sp", tag="sp")
                            nc.tensor.matmul(
                                sp[:, 0:w], kp[:, tk * D:(tk + 1) * D],
                                qp[:, ti0 + off:ti0 + TCH],
                                start=True, stop=True)
                            att = attp.tile([D, TCH], BF16, name="att",
                                            tag="att")
                            nc.scalar.activation(att[:, 0:w], sp[:, 0:w],
                                                 AF.Exp, bias=0.0, scale=scale)
                            if j >= 0:
                                nc.vector.tensor_mul(
                                    att[:, 0:D], att[:, 0:D], tri_t[:])
                            att_tiles[i] = att

                        def emit_av(i):
                            ic, tk, j = work[i]
                            w = TCH if j < 0 else TCH - j * D
                            off = 0 if j < 0 else j * D
                            first = (j == 0 if ic == 0 else tk == 0)
                            last = (j == NDB - 1)
                            if first:
                                po_t[ic] = ps_o.tile([D, TCH], F32, name="po",
                                                     tag="po")
                                pd_t[ic] = ps_d.tile([D, TCH], F32, name="pd",
                                                     tag="pd")
                            att = att_tiles[i]
                            nc.tensor.matmul(
                                po_t[ic][:, off:TCH],
                                v_sb[tk][:, h * D:(h + 1) * D], att[:, 0:w],
                                start=first, stop=last,
                                skip_group_check=True)
                            nc.tensor.matmul(
                                pd_t[ic][:, off:TCH], ones_sq[:], att[:, 0:w],
                                start=first, stop=last,
                                skip_group_check=True)
                            att_tiles[i] = None
                            if last:
                                ti0 = ic * TCH
                                rec = normp.tile([D, TCH], F32, name="rec",
                                                 tag="rec")
                                nc.vector.reciprocal_approx_fast(
                                    rec[:], pd_t[ic][:])
                                nc.vector.tensor_mul(
                                    yt[:, ti0:ti0 + TCH], po_t[ic][:], rec[:])

                        emit_s(0)
                        if n > 1:
                            emit_s(1)
                        for i in range(n):
                            if i + 2 < n:
                                emit_s(i + 2)
                            emit_av(i)
                    nc.sync.dma_start(y_dram[h, :, :], yt[:])
                    # exchange heads as soon as staged: pair-groups for
                    # h0-5, per-head for h6/h7 so the final exchanges are
                    # small and overlap the proj phase start.
                    if h in (1, 3, 5):
                        g = h // 2
                        nc.gpsimd.collective_compute(
                            "AllGather",
                            mybir.AluOpType.bypass,
                            ins=[y_dram[2 * g:2 * g + 2, :, :]],
                            outs=[ag_out[g, :, :, :, :]],
                            replica_groups=PAIRS,
                        )
                        # stage groups 0-1 into SBUF now (group 2 and the
                        # head-6/7 exchanges are staged at proj start, when
                        # the xT tiles have been freed). Rank order is
                        # core-id order: identical layout on both cores.
                        if g < 2:
                            ypt = ypa.tile([D, 4 * T], BF16, name=f"agg{g}")
                            nc.sync.dma_start(
                                ypt[:],
                                ag_out[g].rearrange("r h p f -> p r h f"))
                            for r in range(2):
                                for hh in range(2):
                                    gh = r * 8 + 2 * g + hh
                                    agy[gh] = ypt[:, (2 * r + hh) * T:
                                                  (2 * r + hh + 1) * T]
                    elif h >= 6:
                        nc.gpsimd.collective_compute(
                            "AllGather",
                            mybir.AluOpType.bypass,
                            ins=[y_dram[h:h + 1, :, :]],
                            outs=[(ag6 if h == 6 else ag7)[:, :, :]],
                            replica_groups=PAIRS,
                        )

            # ---------------- Phase P: proj ----------------
            yres_ctx.__exit__(None, None, None)
            qkst_ctx.__exit__(None, None, None)
            wqkp_ctx.__exit__(None, None, None)
            xtp_ctx.__exit__(None, None, None)
            vres_ctx.__exit__(None, None, None)
            with (
                tc.tile_pool(name="ypb", bufs=1) as ypb,
                tc.tile_pool(name="wpp", bufs=1) as wpp,
                tc.tile_pool(name="pp", bufs=4, space="PSUM") as ppp,
                tc.tile_pool(name="post", bufs=3) as post,
            ):
                # wp weights stream per-chunk on the SP queue so the first
                # contraction chain is paced, not blocked. The remaining
                # head staging (group 2, then heads 6/7) rides the
                # Activation queue — idle by now — in parallel, ordered so
                # the AG7-gated transfer is last and blocks nothing.
                bp_t = constp.tile([D, COLS], F32, name="bp_t")
                nc.scalar.dma_start(bp_t[:], bpb[:, :])
                for r in range(2):
                    for hh in range(2):
                        gh = r * 8 + 4 + hh
                        ypt = ypb.tile([D, T], BF16, name=f"ag{gh}")
                        nc.scalar.dma_start(ypt[:], ag_out[2, r, hh, :, :])
                        agy[gh] = ypt
                for hsrc, agt in ((6, ag6), (7, ag7)):
                    for r in range(2):
                        gh = r * 8 + hsrc
                        ypt = ypb.tile([D, T], BF16, name=f"ag{gh}")
                        nc.scalar.dma_start(ypt[:], agt[r, :, :])
                        agy[gh] = ypt
                wp_t = []
                for c in range(NC_CH):
                    t = wpp.tile([D, COLS], BF16, name=f"wp{c}")
                    wp_t.append(t)
                for c in range(NC_CH):
                    nc.sync.dma_start(wp_t[c][:], wp[c * D:(c + 1) * D, :])
                ysrc = [agy[gh] for gh in PROJ_ORDER]
                for tb in range(NTB):
                    ps = [ppp.tile([D, 512], F32, name="pp", tag="pp")
                          for _ in range(NPR)]
                    for ci in range(NC_CH):
                        for nr in range(NPR):
                            nc.tensor.matmul(
                                ps[nr][:], ysrc[ci][:, tb * D:(tb + 1) * D],
                                wp_t[ci][:, nr * 512:(nr + 1) * 512],
                                start=(ci == 0), stop=(ci == NC_CH - 1))
                    st = post.tile([D, COLS], F32, name="pst", tag="pst")
                    for nr in range(NPR):
                        nc.vector.tensor_add(
                            st[:, nr * 512:(nr + 1) * 512], ps[nr][:],
                            bp_t[:, nr * 512:(nr + 1) * 512])
                    nc.sync.dma_start(out_ext[tb * D:(tb + 1) * D, :], st[:])
            ypa_ctx.__exit__(None, None, None)
    nc.finalize()
    return nc


def _prep_inputs(cfg: Cfg, x, w_attn, b_attn, w_proj, b_proj):
    """Host-side shard/cast. Returns in_maps (list of dicts per core)."""
    T, C, CP, HPC = cfg.T, cfg.C, cfg.CP, cfg.HPC
    bf = ml_dtypes.bfloat16
    qscale = 1.0 / math.sqrt(D)
    wq = w_attn[:, 0:C] * qscale
    wk = w_attn[:, C:2 * C]
    wvf = w_attn[:, 2 * C:3 * C]
    bq = b_attn[0:C] * qscale
    bk, bvf = b_attn[C:2 * C], b_attn[2 * C:3 * C]

    # 0/1 lower-triangle keep mask for the diagonal 128-block:
    # att[p=t_k, f=t_q] visible iff f >= p
    f = np.arange(D)[None, :]
    p = np.arange(D)[:, None]
    tri = (f >= p).astype(bf)

    in_maps = []
    for core in range(N_CORES):
        b = core // 2
        g = core % 2
        h0 = g * HPC * D            # first col of this head group
        sl = slice(h0, h0 + CP)
        xTc = np.ascontiguousarray(x[b].T).astype(bf)
        wqk_cols = []
        for h in range(HPC):
            hs = slice(h0 + h * D, h0 + (h + 1) * D)
            wqk_cols.append(wq[:, hs])
            wqk_cols.append(wk[:, hs])
        wqk_c = np.concatenate(wqk_cols, axis=1).astype(bf)
        wv_c = wvf[:, sl].astype(bf)
        # proj: rows in PROJ_ORDER of global heads; cols = core's half
        csl = slice(g * (C // 2), (g + 1) * (C // 2))
        wp_c = np.concatenate(
            [w_proj[gh * D:(gh + 1) * D, csl] for gh in PROJ_ORDER],
            axis=0).astype(bf)
        bqk_cols = []
        for h in range(HPC):
            hs = slice(h0 + h * D, h0 + (h + 1) * D)
            bqk_cols.append(bq[hs])
            bqk_cols.append(bk[hs])
        bqk_c = np.ascontiguousarray(np.stack(bqk_cols, axis=1)).astype(
            np.float32)
        in_maps.append({
            "xT": xTc,
            "wqk": wqk_c,
            "wv": wv_c,
            "wp": wp_c,
            "bqk": bqk_c,
            "bvb": np.broadcast_to(
                bvf[sl].astype(np.float32), (D, CP)).copy(),
            "bpb": np.broadcast_to(
                b_proj[csl].astype(np.float32), (D, C // 2)).copy(),
            "tri": tri,
        })
    return in_maps


_CFG = Cfg()


def kernel(x, w_attn, b_attn, w_proj, b_proj, _trace=False, _cfg=None):
    from concourse.bass_utils import run_bass_kernel_spmd
    cfg = _cfg or _CFG
    x = np.asarray(x, dtype=np.float32)
    w_attn = np.asarray(w_attn, dtype=np.float32)
    b_attn = np.asarray(b_attn, dtype=np.float32)
    w_proj = np.asarray(w_proj, dtype=np.float32)
    b_proj = np.asarray(b_proj, dtype=np.float32)

    in_maps = _prep_inputs(cfg, x, w_attn, b_attn, w_proj, b_proj)
    nc = build_kernel(cfg)
    res = run_bass_kernel_spmd(nc, in_maps, list(range(N_CORES)), trace=_trace)
    outs = []
    for b in range(cfg.B):
        left = res.results[2 * b]["out"]
        right = res.results[2 * b + 1]["out"]
        outs.append(np.concatenate([left, right], axis=1))
    full = np.stack(outs, axis=0).astype(np.float32)
    if _trace:
        kernel.last_exec_time_ns = res.exec_time_ns
        kernel.last_mean_exec_time_ns = res.mean_exec_time_ns
        kernel.last_scope_times = res.per_core_scope_times
    return full
